# revision 8
# baseline (speedup 1.0000x reference)
"""MoE gate (nn_MoEGate) Trainium2 Bass kernel.

reference semantics (jax):
    logits = einsum("bsd,ed->bse", x, W_gate)          # [4,4096,2048]x[64,2048]
    scores = softmax(logits, -1)
    topk_w, topk_i = top_k(scores, 8); topk_w /= sum(topk_w)+eps
    aux_loss = seq-level load-balance loss (scalar)

Strategy: data-parallel over 8 NeuronCores, 2048 tokens per core (each core
holds a contiguous half of one batch row, so the per-batch aux partials
combine cleanly on host). Host pre-transposes each token shard to
[d=2048, t=2048] so the gate matmul's contraction dim lands on SBUF
partitions with fully contiguous DMA runs.

Per core:
  - 4 groups of 512 tokens; one 4MB DMA per group ([128, 16 dchunk, 512 tok]).
  - per 128-token tile: 16 fp32 matmuls accumulate PSUM logits [128t, 64e];
    top-8 ranked on raw logits via DVE max/max_index (matches jax top_k
    ordering incl. ties); softmax via ACT Exp with accumulated row-sum;
    weights = exp(top8 - max) / sum via DVE divide;
    aux partials via two [1,64]-accumulating matmuls (recipZ / ones as lhsT).
  - outputs: idx u32 [128,16,8], w f32 [128,16,8], aux f32 [2,64].

Host finalizes: reshape topk outputs, combine per-core aux partials into the
scalar aux loss.
"""

import sys

import numpy as np

if "/opt/trn_rl_repo" not in sys.path:  # concourse is importable in-container
    sys.path.insert(0, "/opt/trn_rl_repo")

import concourse.bass as bass
import concourse.tile as tile
from concourse import bacc, mybir
from concourse.bass_utils import run_bass_kernel_spmd

N_CORES = 8
B, S, D, E, TOPK = 4, 4096, 2048, 64, 8
T = (B * S) // N_CORES  # tokens per core = 2048
P = 128  # partitions
ND = D // P  # 16 contraction chunks
NT = T // P  # 16 token tiles per core
GROUP_T = 512  # tokens per DMA group
NG = T // GROUP_T  # 4 groups
TILES_PER_G = GROUP_T // P  # 4

F32 = mybir.dt.float32
U32 = mybir.dt.uint32
AX = mybir.AxisListType
ALU = mybir.AluOpType
ACT_FN = mybir.ActivationFunctionType

_BUILT = None  # cached (nc,) so repeated kernel() calls reuse the program


def _build():
    nc = bacc.Bacc("TRN2", target_bir_lowering=False, debug=False)

    xT = nc.dram_tensor("xT", [D, T], F32, kind="ExternalInput")
    wT = nc.dram_tensor("wT", [D, E], F32, kind="ExternalInput")
    out_idx = nc.dram_tensor("out_idx", [P, NT, TOPK], U32, kind="ExternalOutput")
    out_w = nc.dram_tensor("out_w", [P, NT, TOPK], F32, kind="ExternalOutput")
    out_aux = nc.dram_tensor("out_aux", [1, 2 * E], F32, kind="ExternalOutput")

    xT_r = xT.ap().rearrange("(j p) t -> p j t", p=P)  # [128, 16, 2048]
    wT_r = wT.ap().rearrange("(j p) e -> p j e", p=P)  # [128, 16, 64]

    with tile.TileContext(nc) as tc:
        with (
            tc.tile_pool(name="const", bufs=1) as const_pool,
            tc.tile_pool(name="xg", bufs=2) as x_pool,
            tc.tile_pool(name="work", bufs=3) as work_pool,
            tc.tile_pool(name="lg_ps", bufs=2, space="PSUM") as ps_pool,
            tc.tile_pool(name="aux_ps", bufs=1, space="PSUM") as aux_ps_pool,
        ):
            w_sb = const_pool.tile([P, ND, E], F32)
            nc.sync.dma_start(w_sb[:], wT_r)
            ones = const_pool.tile([P, 1], F32)
            nc.vector.memset(ones[:], 1.0)

            stage_idx = const_pool.tile([P, NT, TOPK], U32)
            stage_w = const_pool.tile([P, NT, TOPK], F32)

            probs_ps = aux_ps_pool.tile([1, E], F32, tag="probs")
            cnt_ps = aux_ps_pool.tile([1, E], F32, tag="cnt")

            for g in range(NG):
                x_sb = x_pool.tile([P, ND, GROUP_T], F32)
                nc.sync.dma_start(
                    x_sb[:], xT_r[:, :, g * GROUP_T : (g + 1) * GROUP_T]
                )
                lg_ps = ps_pool.tile([P, TILES_PER_G, E], F32)
                for ti in range(TILES_PER_G):
                    i = g * TILES_PER_G + ti  # global tile index
                    lg = lg_ps[:, ti, :]
                    for j in range(ND):
                        nc.tensor.matmul(
                            lg,
                            lhsT=x_sb[:, j, ti * P : (ti + 1) * P],
                            rhs=w_sb[:, j, :],
                            start=(j == 0),
                            stop=(j == ND - 1),
                        )

                    # SBUF copy of logits for the DVE top-8 ops
                    lg_sb = work_pool.tile([P, E], F32, tag="lg_sb")
                    nc.scalar.copy(lg_sb[:], lg)

                    negmax = work_pool.tile([P, 1], F32, tag="negmax")
                    nc.vector.reduce_max(negmax[:], lg, axis=AX.X, negate=True)

                    lmax8 = work_pool.tile([P, TOPK], F32, tag="lmax8")
                    nc.vector.max(out=lmax8[:], in_=lg_sb[:])
                    nc.vector.max_index(
                        out=stage_idx[:, i, :], in_max=lmax8[:], in_values=lg_sb[:]
                    )

                    # exps = exp(l - max), Z = row sum (ACT accumulate)
                    exps = work_pool.tile([P, E], F32, tag="exps")
                    zsum = work_pool.tile([P, 1], F32, tag="zsum")
                    nc.scalar.activation(
                        exps[:], lg_sb[:], ACT_FN.Exp,
                        bias=negmax[:], accum_out=zsum[:],
                    )
                    e8 = work_pool.tile([P, TOPK], F32, tag="e8")
                    nc.scalar.activation(e8[:], lmax8[:], ACT_FN.Exp, bias=negmax[:])

                    s8 = work_pool.tile([P, 1], F32, tag="s8")
                    nc.vector.reduce_sum(s8[:], e8[:], axis=AX.X)
                    r8 = work_pool.tile([P, 1], F32, tag="r8")
                    nc.vector.reciprocal(r8[:], s8[:])
                    nc.vector.tensor_scalar_mul(stage_w[:, i, :], e8[:], r8[:])

                    # aux partials: probs += (1/Z)^T @ exps ; cnt += 1^T @ top1mask
                    rz = work_pool.tile([P, 1], F32, tag="rz")
                    nc.vector.reciprocal(rz[:], zsum[:])
                    mask = work_pool.tile([P, E], F32, tag="mask")
                    nc.vector.tensor_scalar(
                        mask[:], lg_sb[:], lmax8[:, 0:1], None, op0=ALU.is_equal
                    )
                    nc.tensor.matmul(
                        probs_ps[:], lhsT=rz[:], rhs=exps[:],
                        start=(i == 0), stop=(i == NT - 1), skip_group_check=True,
                    )
                    nc.tensor.matmul(
                        cnt_ps[:], lhsT=ones[:], rhs=mask[:],
                        start=(i == 0), stop=(i == NT - 1), skip_group_check=True,
                    )

            aux_sb = const_pool.tile([1, 2 * E], F32)
            nc.vector.tensor_copy(aux_sb[:, 0:E], probs_ps[:])
            nc.vector.tensor_copy(aux_sb[:, E : 2 * E], cnt_ps[:])

            nc.sync.dma_start(out_idx.ap(), stage_idx[:])
            nc.sync.dma_start(out_w.ap(), stage_w[:])
            nc.sync.dma_start(out_aux.ap(), aux_sb[:])

    nc.compile()
    return nc


def _get_nc():
    global _BUILT
    if _BUILT is None:
        _BUILT = _build()
    return _BUILT


def _make_in_maps(x, W_gate):
    xf = np.ascontiguousarray(np.asarray(x, dtype=np.float32)).reshape(B * S, D)
    wTn = np.ascontiguousarray(np.asarray(W_gate, dtype=np.float32).T)  # [D, E]
    in_maps = []
    for c in range(N_CORES):
        shard = np.ascontiguousarray(xf[c * T : (c + 1) * T, :].T)  # [D, T]
        in_maps.append({"xT": shard, "wT": wTn})
    return in_maps


def _finalize(results):
    idx_parts, w_parts, probs_parts, cnt_parts = [], [], [], []
    for r in results:
        # [P, NT, K] partition-major -> [T, K] token-major (t = ti*128 + p)
        idx_parts.append(
            np.transpose(r["out_idx"], (1, 0, 2)).reshape(T, TOPK).astype(np.int32)
        )
        w_parts.append(np.transpose(r["out_w"], (1, 0, 2)).reshape(T, TOPK))
        probs_parts.append(r["out_aux"][0, :E])
        cnt_parts.append(r["out_aux"][0, E:])

    topk_idx = np.concatenate(idx_parts, axis=0)
    topk_w = np.ascontiguousarray(np.concatenate(w_parts, axis=0))

    # two cores per batch row
    probs = np.stack(probs_parts).reshape(B, 2, E).sum(axis=1, dtype=np.float32)
    cnt = np.stack(cnt_parts).reshape(B, 2, E).sum(axis=1, dtype=np.float32)
    probs = probs / (probs.sum(axis=-1, keepdims=True) + np.float32(1e-9))
    cnt = cnt / (cnt.sum(axis=-1, keepdims=True) + np.float32(1e-9))
    aux = (probs * cnt).sum(axis=-1).mean() * np.float32(E * 0.01)
    return topk_idx, topk_w, np.float32(aux)


def kernel(x, W_gate):
    nc = _get_nc()
    in_maps = _make_in_maps(x, W_gate)
    res = run_bass_kernel_spmd(nc, in_maps, list(range(N_CORES))).results
    return _finalize(res)


# revision 32
# speedup vs baseline: 26.0571x; 26.0571x over previous
"""MoE gate (nn_MoEGate) Trainium2 Bass kernel.

reference semantics (jax):
    logits = einsum("bsd,ed->bse", x, W_gate)          # [4,4096,2048]x[64,2048]
    scores = softmax(logits, -1)
    topk_w, topk_i = top_k(scores, 8); topk_w /= sum(topk_w)+eps
    aux_loss = seq-level load-balance loss (scalar)

Strategy: data-parallel over 8 NeuronCores, 2048 tokens per core (each core
holds a contiguous half of one batch row, so the per-batch aux partials
combine cleanly on host). Host pre-transposes each token shard to
[d=2048, t=2048] so the gate matmul's contraction dim lands on SBUF
partitions with fully contiguous DMA runs.

Per core:
  - 9 token groups (7x256 + 2x128 tokens; the smaller final groups shrink the
    compute tail left after the last input byte lands); one DMA per group.
  - per 128-token tile: 16 fp32 matmuls accumulate a dedicated PSUM bank of
    logits [128t, 64e] (per-tile banks keep PE writes from serializing
    against the previous tile's readers);
    top-8 ranked on raw logits via DVE max/max_index (matches jax top_k
    ordering incl. ties); softmax via ACT Exp with accumulated row-sum;
    weights = exp(top8 - max) * recip(sum) on DVE.
  - aux-loss partials: per-tile [exps|top1mask] (rhs) and [recipZ|ones]
    (lhsT) are staged; accumulating [128,2]x[128,128] matmuls run one group
    behind the stream so PE never stalls on the DVE/ACT chain; result
    [probs_partial | cnt_partial] lands in one PSUM tile.
  - outputs: idx/w for tiles 0..14 flush during the last input load; tile 15
    idx/w + aux ride a single fused tail DMA.

Host finalizes: reshape topk outputs, combine per-core aux partials into the
scalar aux loss.
"""

import sys

import numpy as np

if "/opt/trn_rl_repo" not in sys.path:  # concourse is importable in-container
    sys.path.insert(0, "/opt/trn_rl_repo")

import concourse.bass as bass
import concourse.tile as tile
from concourse import bacc, mybir
from concourse.bass_utils import run_bass_kernel_spmd

N_CORES = 8
B, S, D, E, TOPK = 4, 4096, 2048, 64, 8
T = (B * S) // N_CORES  # tokens per core = 2048
P = 128  # partitions
ND = D // P  # 16 contraction chunks
NT = T // P  # 16 token tiles per core
GROUP_TILES = [2, 2, 2, 2, 2, 2, 2, 1, 1]  # token tiles per DMA group
assert sum(GROUP_TILES) * P == T
TAIL_COLS = 2 * TOPK + 2 * E  # fused tail: idx15 | w15 | aux [2,128]

F32 = mybir.dt.float32
U32 = mybir.dt.uint32
AX = mybir.AxisListType
ALU = mybir.AluOpType
ACT_FN = mybir.ActivationFunctionType

_BUILT = None  # cached so repeated kernel() calls reuse the compiled program


def _build(reps=1, stage="full"):
    """Build the per-core program. reps>1 repeats the whole body in-NEFF
    (benchmarking only). stage: "full" | "dma" | "mm" (bench variants)."""
    nc = bacc.Bacc("TRN2", target_bir_lowering=False, debug=False)

    xT = nc.dram_tensor("xT", [D, T], F32, kind="ExternalInput")
    # host-packed gate weight: wP[p, j*64+e] = W_gate[e, j*128+p] -> fully
    # contiguous per-partition DMA
    wP = nc.dram_tensor("wP", [P, ND * E], F32, kind="ExternalInput")
    out_idx = nc.dram_tensor("out_idx", [P, NT - 1, TOPK], U32, kind="ExternalOutput")
    out_w = nc.dram_tensor("out_w", [P, NT - 1, TOPK], F32, kind="ExternalOutput")
    out_tail = nc.dram_tensor("out_tail", [P, TAIL_COLS], F32, kind="ExternalOutput")

    xT_r = xT.ap().rearrange("(j p) t -> p j t", p=P)  # [128, 16, 2048]

    with tile.TileContext(nc) as tc:
        with (
            tc.tile_pool(name="const", bufs=1) as const_pool,
            tc.tile_pool(name="xg", bufs=6) as x_pool,
            tc.tile_pool(name="work", bufs=4) as work_pool,
            tc.tile_pool(name="lg_ps", bufs=6, space="PSUM") as ps_pool,
            tc.tile_pool(name="aux_ps", bufs=1, space="PSUM") as aux_ps_pool,
        ):
            w_sb = const_pool.tile([P, ND, E], F32)
            nc.sync.dma_start(w_sb[:], wP.ap())

            stage_idx = const_pool.tile([P, NT - 1, TOPK], U32)
            stage_w = const_pool.tile([P, NT - 1, TOPK], F32)
            stage_tail = const_pool.tile([P, TAIL_COLS], F32)
            nc.vector.memset(stage_tail[:], 0.0)
            # packed per-tile aux operands: rhs = [exps | mask], lhsT = [rz | 1]
            em_all = const_pool.tile([P, NT, 2 * E], F32)
            rzo_all = const_pool.tile([P, NT, 2], F32)

            for rep in range(reps):
                if stage == "full":
                    nc.vector.memset(rzo_all[:], 1.0)
                aux_ps = None
                if stage == "full":
                    aux_ps = aux_ps_pool.tile([2, 2 * E], F32, tag="aux_ps")

                def aux_mm(i):
                    # [probs | .] row 0, [. | cnt] row 1, accumulated over tiles
                    nc.tensor.matmul(
                        aux_ps[:],
                        lhsT=rzo_all[:, i, :],
                        rhs=em_all[:, i, :],
                        start=(i == 0),
                        stop=(i == NT - 1),
                        skip_group_check=True,
                    )

                tok0 = 0
                prev_tiles = []  # tiles whose aux matmul is still pending
                for g, gt in enumerate(GROUP_TILES):
                    group_t = gt * P
                    x_sb = x_pool.tile([P, ND, group_t], F32, tag="x_sb")
                    if g == len(GROUP_TILES) - 1:
                        # split the final load so its first-half matmuls can
                        # start while the second half is still in flight
                        nc.sync.dma_start(
                            x_sb[:, 0 : ND // 2, :],
                            xT_r[:, 0 : ND // 2, tok0 : tok0 + group_t],
                        )
                        nc.sync.dma_start(
                            x_sb[:, ND // 2 : ND, :],
                            xT_r[:, ND // 2 : ND, tok0 : tok0 + group_t],
                        )
                    else:
                        nc.sync.dma_start(
                            x_sb[:], xT_r[:, :, tok0 : tok0 + group_t]
                        )
                    if stage == "dma":
                        sink = work_pool.tile([P, 1], F32, tag="sink")
                        nc.vector.tensor_copy(sink[:], x_sb[:, 0, 0:1])
                        tok0 += group_t
                        continue
                    for ti in range(gt):
                        i = tok0 // P + ti  # global tile index
                        lg_ps = ps_pool.tile([P, E], F32)
                        for j in range(ND):
                            nc.tensor.matmul(
                                lg_ps[:],
                                lhsT=x_sb[:, j, ti * P : (ti + 1) * P],
                                rhs=w_sb[:, j, :],
                                start=(j == 0),
                                stop=(j == ND - 1),
                            )

                        if stage == "mm":
                            sink = work_pool.tile([P, 1], F32, tag="sink")
                            nc.vector.tensor_copy(sink[:], lg_ps[:, 0:1])
                            continue

                        idx_dst = (
                            stage_idx[:, i, :] if i < NT - 1
                            else stage_tail[:, 0:TOPK].bitcast(U32)
                        )
                        w_dst = (
                            stage_w[:, i, :] if i < NT - 1
                            else stage_tail[:, TOPK : 2 * TOPK]
                        )

                        # SBUF copy of logits for the DVE top-8 ops
                        lg_sb = work_pool.tile([P, E], F32, tag="lg_sb")
                        nc.scalar.copy(lg_sb[:], lg_ps[:])
                        negmax = work_pool.tile([P, 1], F32, tag="negmax")
                        nc.vector.reduce_max(
                            negmax[:], lg_ps[:], axis=AX.X, negate=True
                        )

                        lmax8 = work_pool.tile([P, TOPK], F32, tag="lmax8")
                        nc.vector.max(out=lmax8[:], in_=lg_sb[:])
                        nc.vector.max_index(
                            out=idx_dst, in_max=lmax8[:], in_values=lg_sb[:]
                        )

                        # exps = exp(l - max) into packed rhs; Z = row sum
                        zsum = work_pool.tile([P, 1], F32, tag="zsum")
                        nc.scalar.activation(
                            em_all[:, i, 0:E], lg_sb[:], ACT_FN.Exp,
                            bias=negmax[:], accum_out=zsum[:],
                        )
                        # top-8 exps + their sum (ACT accumulate), weights on DVE
                        e8 = work_pool.tile([P, TOPK], F32, tag="e8")
                        s8 = work_pool.tile([P, 1], F32, tag="s8")
                        nc.scalar.activation(
                            e8[:], lmax8[:], ACT_FN.Exp,
                            bias=negmax[:], accum_out=s8[:],
                        )
                        r8 = work_pool.tile([P, 1], F32, tag="r8")
                        nc.vector.reciprocal(r8[:], s8[:])
                        nc.vector.tensor_scalar_mul(w_dst, e8[:], r8[:])

                        # aux lhsT column 0 = 1/Z ; top-1 mask into packed rhs
                        nc.vector.reciprocal(rzo_all[:, i, 0:1], zsum[:])
                        nc.vector.tensor_scalar(
                            em_all[:, i, E : 2 * E], lg_sb[:], lmax8[:, 0:1],
                            None, op0=ALU.is_equal,
                        )

                    # aux matmuls run one group behind so PE never stalls on
                    # the DVE/ACT chain of the tile they consume
                    if stage == "full":
                        for i in prev_tiles:
                            aux_mm(i)
                        prev_tiles = [tok0 // P + ti for ti in range(gt)]
                        if tok0 // P + gt == NT - 1:
                            # tiles 0..14 done: flush their outputs while the
                            # last input group is still in flight
                            nc.sync.dma_start(out_idx.ap(), stage_idx[:])
                            nc.sync.dma_start(out_w.ap(), stage_w[:])
                    tok0 += group_t

                if stage == "full":
                    for i in prev_tiles:
                        aux_mm(i)
                    nc.vector.tensor_copy(
                        stage_tail[0:2, 2 * TOPK : TAIL_COLS], aux_ps[:]
                    )
                else:
                    nc.vector.memset(stage_tail[:], 0.0)
                    nc.vector.memset(stage_idx[:], 0)
                    nc.vector.memset(stage_w[:], 0.0)
                    nc.sync.dma_start(out_idx.ap(), stage_idx[:])
                    nc.sync.dma_start(out_w.ap(), stage_w[:])

            nc.sync.dma_start(out_tail.ap(), stage_tail[:])

    nc.compile()
    return nc


def _get_nc():
    global _BUILT
    if _BUILT is None:
        _BUILT = _build()
    return _BUILT


def _make_in_maps(x, W_gate):
    xf = np.ascontiguousarray(np.asarray(x, dtype=np.float32)).reshape(B * S, D)
    # wP[p, j*E+e] = W_gate[e, j*P+p]
    wPn = np.ascontiguousarray(
        np.asarray(W_gate, dtype=np.float32).T.reshape(ND, P, E)
        .transpose(1, 0, 2).reshape(P, ND * E)
    )
    in_maps = []
    for c in range(N_CORES):
        shard = np.ascontiguousarray(xf[c * T : (c + 1) * T, :].T)  # [D, T]
        in_maps.append({"xT": shard, "wP": wPn})
    return in_maps


def _finalize(results):
    idx_parts, w_parts, probs_parts, cnt_parts = [], [], [], []
    for r in results:
        tail = r["out_tail"]  # [P, 16+128] f32
        idx15 = tail[:, 0:TOPK].view(np.uint32)[:, None, :]  # [P,1,K]
        w15 = tail[:, TOPK : 2 * TOPK][:, None, :]
        aux = tail[0:2, 2 * TOPK :]  # [2, 128]
        idx_full = np.concatenate([r["out_idx"], idx15], axis=1)  # [P, NT, K]
        w_full = np.concatenate([r["out_w"], w15], axis=1)
        # [P, NT, K] partition-major -> [T, K] token-major (t = ti*128 + p)
        idx_parts.append(
            np.transpose(idx_full, (1, 0, 2)).reshape(T, TOPK).astype(np.int32)
        )
        w_parts.append(np.transpose(w_full, (1, 0, 2)).reshape(T, TOPK))
        probs_parts.append(aux[0, :E])
        cnt_parts.append(aux[1, E:])

    topk_idx = np.concatenate(idx_parts, axis=0)
    topk_w = np.ascontiguousarray(np.concatenate(w_parts, axis=0))

    # two cores per batch row
    probs = np.stack(probs_parts).reshape(B, 2, E).sum(axis=1, dtype=np.float32)
    cnt = np.stack(cnt_parts).reshape(B, 2, E).sum(axis=1, dtype=np.float32)
    probs = probs / (probs.sum(axis=-1, keepdims=True) + np.float32(1e-9))
    cnt = cnt / (cnt.sum(axis=-1, keepdims=True) + np.float32(1e-9))
    aux = (probs * cnt).sum(axis=-1).mean() * np.float32(E * 0.01)
    return topk_idx, topk_w, np.float32(aux)


def kernel(x, W_gate):
    nc = _get_nc()
    in_maps = _make_in_maps(x, W_gate)
    res = run_bass_kernel_spmd(nc, in_maps, list(range(N_CORES))).results
    return _finalize(res)


# revision 41
# speedup vs baseline: 51.4652x; 1.9751x over previous
"""MoE gate (nn_MoEGate) Trainium2 Bass kernel.

reference semantics (jax):
    logits = einsum("bsd,ed->bse", x, W_gate)          # [4,4096,2048]x[64,2048]
    scores = softmax(logits, -1)
    topk_w, topk_i = top_k(scores, 8); topk_w /= sum(topk_w)+eps
    aux_loss = seq-level load-balance loss (scalar)

Strategy: data-parallel over 8 NeuronCores, 2048 tokens per core (each core
holds a contiguous half of one batch row, so the per-batch aux partials
combine cleanly on host).

Precision/speed trick: trn2's native fp32 matmul self-loads its weights and
runs 2 passes (~8x the bf16 cost, weight load not overlappable). Instead the
host splits x and (64x-scaled) W_gate into fp16 hi+lo pairs — same total HBM
bytes as fp32 — and the kernel computes
    64*logits = xH*wH + xH*wL + xL*wH    (xL*wL ~ 2^-33, dropped)
with three fp16 matmuls per d-chunk (1 cyc/row, hardware fast-weight-load,
fully pipelined). Combined representation error ~2^-22 — at the same scale
as fp32 matmul rounding itself. The ACT copy that moves logits from PSUM
applies the 1/64 rescale for free.

Per core:
  - token groups ([4,4,4,2,1,1] tiles of 128; smaller final groups shrink
    the compute tail after the last input byte); two fp16 DMAs (hi/lo) per
    group; host pre-transposes shards to [d, t] so the contraction dim lands
    on partitions with contiguous runs.
  - per 128-token tile: 48 fp16 matmuls accumulate logits*64 in a dedicated
    PSUM bank [128t, 64e]; top-8 ranked via DVE max/max_index on the rescaled
    SBUF logits (matches jax top_k ordering incl. ties); softmax via ACT Exp
    with accumulated row-sum; weights = exp(top8-max) * recip(sum8) on DVE.
  - aux-loss partials: per-tile [exps|top1mask] (rhs) and [recipZ|ones]
    (lhsT) staged; accumulating fp32 [128,2]x[128,128] matmuls run one group
    behind the stream so PE never stalls on the DVE/ACT chain.
  - outputs: idx/w for tiles 0..14 flush before the final tile's chain ends;
    tile 15 idx/w + aux ride a single fused tail DMA.

Host finalizes: reshape topk outputs, combine per-core aux partials into the
scalar aux loss.
"""

import sys

import numpy as np

if "/opt/trn_rl_repo" not in sys.path:  # concourse is importable in-container
    sys.path.insert(0, "/opt/trn_rl_repo")

import concourse.bass as bass
import concourse.tile as tile
from concourse import bacc, mybir
from concourse.bass_utils import run_bass_kernel_spmd

N_CORES = 8
B, S, D, E, TOPK = 4, 4096, 2048, 64, 8
T = (B * S) // N_CORES  # tokens per core = 2048
P = 128  # partitions
ND = D // P  # 16 contraction chunks
NT = T // P  # 16 token tiles per core
GROUP_TILES = [4, 4, 4, 2, 1, 1]  # token tiles per DMA group
assert sum(GROUP_TILES) * P == T
TAIL_COLS = 2 * TOPK + 2 * E  # fused tail: idx15 | w15 | aux [2,128]
WSCALE = 64.0  # host pre-scales W so its fp16 lo-half stays normal

F32 = mybir.dt.float32
F16 = mybir.dt.float16
U32 = mybir.dt.uint32
AX = mybir.AxisListType
ALU = mybir.AluOpType
ACT_FN = mybir.ActivationFunctionType

_BUILT = None  # cached so repeated kernel() calls reuse the compiled program


def _build(reps=1, stage="full"):
    """Build the per-core program. reps>1 repeats the whole body in-NEFF
    (benchmarking only). stage: "full" | "dma" | "mm" (bench variants)."""
    nc = bacc.Bacc("TRN2", target_bir_lowering=False, debug=False)

    xH = nc.dram_tensor("xH", [D, T], F16, kind="ExternalInput")
    xL = nc.dram_tensor("xL", [D, T], F16, kind="ExternalInput")
    # host-packed scaled gate weight halves: w?[p, j*64+e] ~ 64*W_gate[e, j*128+p]
    wH = nc.dram_tensor("wH", [P, ND * E], F16, kind="ExternalInput")
    wL = nc.dram_tensor("wL", [P, ND * E], F16, kind="ExternalInput")
    out_idx = nc.dram_tensor("out_idx", [P, NT - 1, TOPK], U32, kind="ExternalOutput")
    out_w = nc.dram_tensor("out_w", [P, NT - 1, TOPK], F32, kind="ExternalOutput")
    out_tail = nc.dram_tensor("out_tail", [P, TAIL_COLS], F32, kind="ExternalOutput")

    xH_r = xH.ap().rearrange("(j p) t -> p j t", p=P)  # [128, 16, 2048]
    xL_r = xL.ap().rearrange("(j p) t -> p j t", p=P)

    with tile.TileContext(nc) as tc:
        with (
            tc.tile_pool(name="const", bufs=1) as const_pool,
            tc.tile_pool(name="xg", bufs=4) as x_pool,
            tc.tile_pool(name="work", bufs=4) as work_pool,
            tc.tile_pool(name="lg_ps", bufs=6, space="PSUM") as ps_pool,
            tc.tile_pool(name="aux_ps", bufs=1, space="PSUM") as aux_ps_pool,
        ):
            wH_sb = const_pool.tile([P, ND, E], F16)
            nc.sync.dma_start(wH_sb[:], wH.ap())
            wL_sb = const_pool.tile([P, ND, E], F16)
            nc.sync.dma_start(wL_sb[:], wL.ap())

            stage_idx = const_pool.tile([P, NT - 1, TOPK], U32)
            stage_w = const_pool.tile([P, NT - 1, TOPK], F32)
            stage_tail = const_pool.tile([P, TAIL_COLS], F32)
            nc.vector.memset(stage_tail[:], 0.0)
            # packed per-tile aux operands: rhs = [exps | mask], lhsT = [rz | 1]
            em_all = const_pool.tile([P, NT, 2 * E], F32)
            rzo_all = const_pool.tile([P, NT, 2], F32)

            for rep in range(reps):
                if stage == "full":
                    nc.vector.memset(rzo_all[:], 1.0)
                aux_ps = None
                if stage == "full":
                    aux_ps = aux_ps_pool.tile([2, 2 * E], F32, tag="aux_ps")

                def aux_mm(i):
                    # [probs | .] row 0, [. | cnt] row 1, accumulated over tiles
                    nc.tensor.matmul(
                        aux_ps[:],
                        lhsT=rzo_all[:, i, :],
                        rhs=em_all[:, i, :],
                        start=(i == 0),
                        stop=(i == NT - 1),
                        skip_group_check=True,
                    )

                tok0 = 0
                prev_tiles = []  # tiles whose aux matmul is still pending
                for g, gt in enumerate(GROUP_TILES):
                    group_t = gt * P
                    xh_sb = x_pool.tile([P, ND, group_t], F16, tag="xh_sb")
                    nc.sync.dma_start(
                        xh_sb[:], xH_r[:, :, tok0 : tok0 + group_t]
                    )
                    xl_sb = x_pool.tile([P, ND, group_t], F16, tag="xl_sb")
                    nc.sync.dma_start(
                        xl_sb[:], xL_r[:, :, tok0 : tok0 + group_t]
                    )
                    if stage == "dma":
                        sink = work_pool.tile([P, 1], F16, tag="sink")
                        nc.vector.tensor_copy(sink[:], xh_sb[:, 0, 0:1])
                        nc.vector.tensor_copy(sink[:], xl_sb[:, 0, 0:1])
                        tok0 += group_t
                        continue
                    for ti in range(gt):
                        i = tok0 // P + ti  # global tile index
                        ts = slice(ti * P, (ti + 1) * P)
                        lg_ps = ps_pool.tile([P, E], F32)
                        for j in range(ND):
                            # 64*logits += xH.wH + xH.wL + xL.wH  (fp16 FWL)
                            nc.tensor.matmul(
                                lg_ps[:], lhsT=xh_sb[:, j, ts], rhs=wH_sb[:, j, :],
                                start=(j == 0), stop=False,
                            )
                            nc.tensor.matmul(
                                lg_ps[:], lhsT=xh_sb[:, j, ts], rhs=wL_sb[:, j, :],
                                start=False, stop=False,
                            )
                            nc.tensor.matmul(
                                lg_ps[:], lhsT=xl_sb[:, j, ts], rhs=wH_sb[:, j, :],
                                start=False, stop=(j == ND - 1),
                            )

                        if stage == "mm":
                            sink = work_pool.tile([P, 1], F16, tag="sink")
                            nc.vector.tensor_copy(
                                sink[:].bitcast(F32), lg_ps[:, 0:1]
                            )
                            continue

                        idx_dst = (
                            stage_idx[:, i, :] if i < NT - 1
                            else stage_tail[:, 0:TOPK].bitcast(U32)
                        )
                        w_dst = (
                            stage_w[:, i, :] if i < NT - 1
                            else stage_tail[:, TOPK : 2 * TOPK]
                        )

                        # rescaled SBUF logits for the DVE top-8 ops
                        lg_sb = work_pool.tile([P, E], F32, tag="lg_sb")
                        nc.scalar.activation(
                            lg_sb[:], lg_ps[:], ACT_FN.Copy, scale=1.0 / WSCALE
                        )
                        negmax = work_pool.tile([P, 1], F32, tag="negmax")
                        nc.vector.reduce_max(
                            negmax[:], lg_sb[:], axis=AX.X, negate=True
                        )

                        lmax8 = work_pool.tile([P, TOPK], F32, tag="lmax8")
                        nc.vector.max(out=lmax8[:], in_=lg_sb[:])
                        nc.vector.max_index(
                            out=idx_dst, in_max=lmax8[:], in_values=lg_sb[:]
                        )

                        # exps = exp(l - max) into packed rhs; Z = row sum
                        zsum = work_pool.tile([P, 1], F32, tag="zsum")
                        nc.scalar.activation(
                            em_all[:, i, 0:E], lg_sb[:], ACT_FN.Exp,
                            bias=negmax[:], accum_out=zsum[:],
                        )
                        # top-8 exps + their sum (ACT accumulate), weights on DVE
                        e8 = work_pool.tile([P, TOPK], F32, tag="e8")
                        s8 = work_pool.tile([P, 1], F32, tag="s8")
                        nc.scalar.activation(
                            e8[:], lmax8[:], ACT_FN.Exp,
                            bias=negmax[:], accum_out=s8[:],
                        )
                        r8 = work_pool.tile([P, 1], F32, tag="r8")
                        nc.vector.reciprocal(r8[:], s8[:])
                        nc.vector.tensor_scalar_mul(w_dst, e8[:], r8[:])

                        # aux lhsT column 0 = 1/Z ; top-1 mask into packed rhs
                        nc.vector.reciprocal(rzo_all[:, i, 0:1], zsum[:])
                        nc.vector.tensor_scalar(
                            em_all[:, i, E : 2 * E], lg_sb[:], lmax8[:, 0:1],
                            None, op0=ALU.is_equal,
                        )

                        if i == NT - 2:
                            # tiles 0..14 done: flush their outputs before the
                            # final tile's chain completes
                            nc.sync.dma_start(out_idx.ap(), stage_idx[:])
                            nc.sync.dma_start(out_w.ap(), stage_w[:])

                    # aux matmuls run one group behind so PE never stalls on
                    # the DVE/ACT chain of the tile they consume
                    if stage == "full":
                        for i in prev_tiles:
                            aux_mm(i)
                        prev_tiles = [tok0 // P + ti for ti in range(gt)]
                    tok0 += group_t

                if stage == "full":
                    for i in prev_tiles:
                        aux_mm(i)
                    nc.vector.tensor_copy(
                        stage_tail[0:2, 2 * TOPK : TAIL_COLS], aux_ps[:]
                    )
                else:
                    nc.vector.memset(stage_tail[:], 0.0)
                    nc.vector.memset(stage_idx[:], 0)
                    nc.vector.memset(stage_w[:], 0.0)
                    nc.sync.dma_start(out_idx.ap(), stage_idx[:])
                    nc.sync.dma_start(out_w.ap(), stage_w[:])

            nc.sync.dma_start(out_tail.ap(), stage_tail[:])

    nc.compile()
    return nc


def _get_nc():
    global _BUILT
    if _BUILT is None:
        _BUILT = _build()
    return _BUILT


def _make_in_maps(x, W_gate):
    xf = np.ascontiguousarray(np.asarray(x, dtype=np.float32)).reshape(B * S, D)
    # scaled weight split: Ws = 64*W ; wH = fp16(Ws) ; wL = fp16(Ws - wH),
    # packed as w[p, j*E+e] = Ws[e, j*P+p]
    Ws = np.asarray(W_gate, dtype=np.float32) * np.float32(WSCALE)
    WsT = np.ascontiguousarray(
        Ws.T.reshape(ND, P, E).transpose(1, 0, 2).reshape(P, ND * E)
    )
    wHn = WsT.astype(np.float16)
    wLn = (WsT - wHn.astype(np.float32)).astype(np.float16)
    in_maps = []
    for c in range(N_CORES):
        shard = np.ascontiguousarray(xf[c * T : (c + 1) * T, :].T)  # [D, T] f32
        sH = shard.astype(np.float16)
        sL = (shard - sH.astype(np.float32)).astype(np.float16)
        in_maps.append({"xH": sH, "xL": sL, "wH": wHn, "wL": wLn})
    return in_maps


def _finalize(results):
    idx_parts, w_parts, probs_parts, cnt_parts = [], [], [], []
    for r in results:
        tail = r["out_tail"]  # [P, 16+128] f32
        idx15 = tail[:, 0:TOPK].view(np.uint32)[:, None, :]  # [P,1,K]
        w15 = tail[:, TOPK : 2 * TOPK][:, None, :]
        aux = tail[0:2, 2 * TOPK :]  # [2, 128]
        idx_full = np.concatenate([r["out_idx"], idx15], axis=1)  # [P, NT, K]
        w_full = np.concatenate([r["out_w"], w15], axis=1)
        # [P, NT, K] partition-major -> [T, K] token-major (t = ti*128 + p)
        idx_parts.append(
            np.transpose(idx_full, (1, 0, 2)).reshape(T, TOPK).astype(np.int32)
        )
        w_parts.append(np.transpose(w_full, (1, 0, 2)).reshape(T, TOPK))
        probs_parts.append(aux[0, :E])
        cnt_parts.append(aux[1, E:])

    topk_idx = np.concatenate(idx_parts, axis=0)
    topk_w = np.ascontiguousarray(np.concatenate(w_parts, axis=0))

    # two cores per batch row
    probs = np.stack(probs_parts).reshape(B, 2, E).sum(axis=1, dtype=np.float32)
    cnt = np.stack(cnt_parts).reshape(B, 2, E).sum(axis=1, dtype=np.float32)
    probs = probs / (probs.sum(axis=-1, keepdims=True) + np.float32(1e-9))
    cnt = cnt / (cnt.sum(axis=-1, keepdims=True) + np.float32(1e-9))
    aux = (probs * cnt).sum(axis=-1).mean() * np.float32(E * 0.01)
    return topk_idx, topk_w, np.float32(aux)


def kernel(x, W_gate):
    nc = _get_nc()
    in_maps = _make_in_maps(x, W_gate)
    res = run_bass_kernel_spmd(nc, in_maps, list(range(N_CORES))).results
    return _finalize(res)


# revision 52
# speedup vs baseline: 54.9260x; 1.0672x over previous
"""MoE gate (nn_MoEGate) Trainium2 Bass kernel.

reference semantics (jax):
    logits = einsum("bsd,ed->bse", x, W_gate)          # [4,4096,2048]x[64,2048]
    scores = softmax(logits, -1)
    topk_w, topk_i = top_k(scores, 8); topk_w /= sum(topk_w)+eps
    aux_loss = seq-level load-balance loss (scalar)

Strategy: data-parallel over 8 NeuronCores, 2048 tokens per core (each core
holds a contiguous half of one batch row, so the per-batch aux partials
combine cleanly on host).

Precision/speed trick: trn2's native fp32 matmul self-loads its weights and
runs 2 passes (~8x the bf16 cost, weight load not overlappable). Instead the
host splits x and (64x-scaled) W_gate into fp16 hi+lo pairs — same total HBM
bytes as fp32 — and the kernel computes
    64*logits = xH*wH + xH*wL + xL*wH    (xL*wL ~ 2^-33, dropped)
with three fp16 matmuls per d-chunk (1 cyc/row, hardware fast-weight-load,
fully pipelined). Combined representation error ~2^-22 — at the same scale
as fp32 matmul rounding itself. The ACT copy that moves logits from PSUM
applies the 1/64 rescale for free.

Per core:
  - token groups ([4,4,4,2,1,1] tiles of 128; smaller final groups shrink
    the compute tail after the last input byte); two fp16 DMAs (hi/lo) per
    group; host pre-transposes shards to [d, t] so the contraction dim lands
    on partitions with contiguous runs.
  - per 128-token tile: 48 fp16 matmuls accumulate logits*64 in a dedicated
    PSUM bank [128t, 64e]; top-8 ranked via DVE max/max_index on the rescaled
    SBUF logits (matches jax top_k ordering incl. ties); softmax via ACT Exp
    with accumulated row-sum; weights = exp(top8-max) * recip(sum8) on DVE.
  - aux-loss partials: per-tile [exps|top1mask] (rhs) and [recipZ|ones]
    (lhsT) staged; accumulating fp32 [128,2]x[128,128] matmuls run one group
    behind the stream so PE never stalls on the DVE/ACT chain.
  - outputs: idx/w for tiles 0..14 flush before the final tile's chain ends;
    tile 15 idx/w + aux ride a single fused tail DMA.

Host finalizes: reshape topk outputs, combine per-core aux partials into the
scalar aux loss.
"""

import sys

import numpy as np

if "/opt/trn_rl_repo" not in sys.path:  # concourse is importable in-container
    sys.path.insert(0, "/opt/trn_rl_repo")

import concourse.bass as bass
import concourse.tile as tile
from concourse import bacc, mybir
from concourse.bass_utils import run_bass_kernel_spmd

N_CORES = 8
B, S, D, E, TOPK = 4, 4096, 2048, 64, 8
T = (B * S) // N_CORES  # tokens per core = 2048
P = 128  # partitions
ND = D // P  # 16 contraction chunks
NT = T // P  # 16 token tiles per core
# token tiles per DMA group: small first group starts the stream sooner
# (descriptor gen gates the first load); small final groups shrink the
# compute tail after the last input byte
GROUP_TILES = [2, 4, 4, 4, 1, 1]
assert sum(GROUP_TILES) * P == T
TAIL_COLS = 2 * TOPK + 2 * E  # fused tail: idx15 | w15 | aux [2,128]
WSCALE = 64.0  # host pre-scales W so its fp16 lo-half stays normal

F32 = mybir.dt.float32
F16 = mybir.dt.float16
U32 = mybir.dt.uint32
AX = mybir.AxisListType
ALU = mybir.AluOpType
ACT_FN = mybir.ActivationFunctionType

_BUILT = None  # cached so repeated kernel() calls reuse the compiled program


def _build(reps=1, stage="full"):
    """Build the per-core program. reps>1 repeats the whole body in-NEFF
    (benchmarking only). stage: "full" | "dma" | "mm" (bench variants)."""
    nc = bacc.Bacc("TRN2", target_bir_lowering=False, debug=False)

    # hi/lo fp16 halves interleaved per token: xP[d, 2t]=hi, xP[d, 2t+1]=lo
    # -> one DMA per token group with 2x-longer contiguous runs
    xP = nc.dram_tensor("xP", [D, 2 * T], F16, kind="ExternalInput")
    # host-packed scaled gate weight halves: w?[p, j*64+e] ~ 64*W_gate[e, j*128+p]
    wH = nc.dram_tensor("wH", [P, ND * E], F16, kind="ExternalInput")
    wL = nc.dram_tensor("wL", [P, ND * E], F16, kind="ExternalInput")
    out_idx = nc.dram_tensor("out_idx", [P, NT - 1, TOPK], U32, kind="ExternalOutput")
    out_w = nc.dram_tensor("out_w", [P, NT - 1, TOPK], F32, kind="ExternalOutput")
    out_tail = nc.dram_tensor("out_tail", [P, TAIL_COLS], F32, kind="ExternalOutput")

    xP_r = xP.ap().rearrange("(j p) v -> p j v", p=P)  # [128, 16, 4096]

    with tile.TileContext(nc) as tc:
        with (
            tc.tile_pool(name="const", bufs=1) as const_pool,
            tc.tile_pool(name="xg", bufs=4) as x_pool,
            tc.tile_pool(name="work", bufs=4) as work_pool,
            tc.tile_pool(name="lg_ps", bufs=6, space="PSUM") as ps_pool,
            tc.tile_pool(name="aux_ps", bufs=1, space="PSUM") as aux_ps_pool,
        ):
            wH_sb = const_pool.tile([P, ND, E], F16)
            nc.sync.dma_start(wH_sb[:], wH.ap())
            wL_sb = const_pool.tile([P, ND, E], F16)
            nc.sync.dma_start(wL_sb[:], wL.ap())

            stage_idx = const_pool.tile([P, NT - 1, TOPK], U32)
            stage_w = const_pool.tile([P, NT - 1, TOPK], F32)
            stage_tail = const_pool.tile([P, TAIL_COLS], F32)
            nc.vector.memset(stage_tail[:], 0.0)
            # packed per-tile aux operands: rhs = [exps | mask], lhsT = [rz | 1]
            em_all = const_pool.tile([P, NT, 2 * E], F32)
            rzo_all = const_pool.tile([P, NT, 2], F32)

            for rep in range(reps):
                if stage == "full":
                    nc.vector.memset(rzo_all[:], 1.0)
                aux_ps = None
                if stage == "full":
                    aux_ps = aux_ps_pool.tile([2, 2 * E], F32, tag="aux_ps")

                def aux_mm(i):
                    # [probs | .] row 0, [. | cnt] row 1, accumulated over tiles
                    nc.tensor.matmul(
                        aux_ps[:],
                        lhsT=rzo_all[:, i, :],
                        rhs=em_all[:, i, :],
                        start=(i == 0),
                        stop=(i == NT - 1),
                        skip_group_check=True,
                    )

                tok0 = 0
                prev_tiles = []  # tiles whose aux matmul is still pending
                for g, gt in enumerate(GROUP_TILES):
                    group_t = gt * P
                    x_sb = x_pool.tile([P, ND, group_t, 2], F16, tag="x_sb")
                    nc.sync.dma_start(
                        x_sb[:], xP_r[:, :, 2 * tok0 : 2 * (tok0 + group_t)]
                    )
                    if stage == "dma":
                        sink = work_pool.tile([P, 1], F16, tag="sink")
                        nc.vector.tensor_copy(sink[:], x_sb[:, 0, 0:1, 0])
                        tok0 += group_t
                        continue
                    for ti in range(gt):
                        i = tok0 // P + ti  # global tile index
                        ts = slice(ti * P, (ti + 1) * P)
                        lg_ps = ps_pool.tile([P, E], F32)
                        for j in range(ND):
                            # 64*logits += xH.wH + xH.wL + xL.wH  (fp16 FWL)
                            nc.tensor.matmul(
                                lg_ps[:], lhsT=x_sb[:, j, ts, 0], rhs=wH_sb[:, j, :],
                                start=(j == 0), stop=False,
                            )
                            nc.tensor.matmul(
                                lg_ps[:], lhsT=x_sb[:, j, ts, 0], rhs=wL_sb[:, j, :],
                                start=False, stop=False,
                            )
                            nc.tensor.matmul(
                                lg_ps[:], lhsT=x_sb[:, j, ts, 1], rhs=wH_sb[:, j, :],
                                start=False, stop=(j == ND - 1),
                            )

                        if stage == "mm":
                            sink = work_pool.tile([P, 1], F16, tag="sink")
                            nc.vector.tensor_copy(
                                sink[:].bitcast(F32), lg_ps[:, 0:1]
                            )
                            continue

                        idx_dst = (
                            stage_idx[:, i, :] if i < NT - 1
                            else stage_tail[:, 0:TOPK].bitcast(U32)
                        )
                        w_dst = (
                            stage_w[:, i, :] if i < NT - 1
                            else stage_tail[:, TOPK : 2 * TOPK]
                        )

                        # rescaled SBUF logits for the DVE top-8 ops
                        lg_sb = work_pool.tile([P, E], F32, tag="lg_sb")
                        nc.scalar.activation(
                            lg_sb[:], lg_ps[:], ACT_FN.Copy, scale=1.0 / WSCALE
                        )
                        negmax = work_pool.tile([P, 1], F32, tag="negmax")
                        nc.vector.reduce_max(
                            negmax[:], lg_sb[:], axis=AX.X, negate=True
                        )

                        lmax8 = work_pool.tile([P, TOPK], F32, tag="lmax8")
                        nc.vector.max(out=lmax8[:], in_=lg_sb[:])
                        nc.vector.max_index(
                            out=idx_dst, in_max=lmax8[:], in_values=lg_sb[:]
                        )

                        # exps = exp(l - max) into packed rhs; Z = row sum
                        zsum = work_pool.tile([P, 1], F32, tag="zsum")
                        nc.scalar.activation(
                            em_all[:, i, 0:E], lg_sb[:], ACT_FN.Exp,
                            bias=negmax[:], accum_out=zsum[:],
                        )
                        # top-8 exps + their sum (ACT accumulate), weights on DVE
                        e8 = work_pool.tile([P, TOPK], F32, tag="e8")
                        s8 = work_pool.tile([P, 1], F32, tag="s8")
                        nc.scalar.activation(
                            e8[:], lmax8[:], ACT_FN.Exp,
                            bias=negmax[:], accum_out=s8[:],
                        )
                        r8 = work_pool.tile([P, 1], F32, tag="r8")
                        nc.vector.reciprocal(r8[:], s8[:])
                        nc.vector.tensor_scalar_mul(w_dst, e8[:], r8[:])

                        # aux lhsT column 0 = 1/Z ; top-1 mask into packed rhs
                        nc.vector.reciprocal(rzo_all[:, i, 0:1], zsum[:])
                        nc.vector.tensor_scalar(
                            em_all[:, i, E : 2 * E], lg_sb[:], lmax8[:, 0:1],
                            None, op0=ALU.is_equal,
                        )

                        if i == NT - 2:
                            # tiles 0..14 done: flush their outputs before the
                            # final tile's chain completes
                            nc.sync.dma_start(out_idx.ap(), stage_idx[:])
                            nc.sync.dma_start(out_w.ap(), stage_w[:])

                    # aux matmuls run one group behind so PE never stalls on
                    # the DVE/ACT chain of the tile they consume
                    if stage == "full":
                        for i in prev_tiles:
                            aux_mm(i)
                        prev_tiles = [tok0 // P + ti for ti in range(gt)]
                    tok0 += group_t

                if stage == "full":
                    for i in prev_tiles:
                        aux_mm(i)
                    nc.vector.tensor_copy(
                        stage_tail[0:2, 2 * TOPK : TAIL_COLS], aux_ps[:]
                    )
                else:
                    nc.vector.memset(stage_tail[:], 0.0)
                    nc.vector.memset(stage_idx[:], 0)
                    nc.vector.memset(stage_w[:], 0.0)
                    nc.sync.dma_start(out_idx.ap(), stage_idx[:])
                    nc.sync.dma_start(out_w.ap(), stage_w[:])

            nc.sync.dma_start(out_tail.ap(), stage_tail[:])

    nc.compile()
    return nc


def _get_nc():
    global _BUILT
    if _BUILT is None:
        _BUILT = _build()
    return _BUILT


def _make_in_maps(x, W_gate):
    xf = np.ascontiguousarray(np.asarray(x, dtype=np.float32)).reshape(B * S, D)
    # scaled weight split: Ws = 64*W ; wH = fp16(Ws) ; wL = fp16(Ws - wH),
    # packed as w[p, j*E+e] = Ws[e, j*P+p]
    Ws = np.asarray(W_gate, dtype=np.float32) * np.float32(WSCALE)
    WsT = np.ascontiguousarray(
        Ws.T.reshape(ND, P, E).transpose(1, 0, 2).reshape(P, ND * E)
    )
    wHn = WsT.astype(np.float16)
    wLn = (WsT - wHn.astype(np.float32)).astype(np.float16)
    in_maps = []
    for c in range(N_CORES):
        shard = np.ascontiguousarray(xf[c * T : (c + 1) * T, :].T)  # [D, T] f32
        sH = shard.astype(np.float16)
        sL = (shard - sH.astype(np.float32)).astype(np.float16)
        xPn = np.ascontiguousarray(
            np.stack([sH, sL], axis=-1).reshape(D, 2 * T)
        )
        in_maps.append({"xP": xPn, "wH": wHn, "wL": wLn})
    return in_maps


def _finalize(results):
    idx_parts, w_parts, probs_parts, cnt_parts = [], [], [], []
    for r in results:
        tail = r["out_tail"]  # [P, 16+128] f32
        idx15 = tail[:, 0:TOPK].view(np.uint32)[:, None, :]  # [P,1,K]
        w15 = tail[:, TOPK : 2 * TOPK][:, None, :]
        aux = tail[0:2, 2 * TOPK :]  # [2, 128]
        idx_full = np.concatenate([r["out_idx"], idx15], axis=1)  # [P, NT, K]
        w_full = np.concatenate([r["out_w"], w15], axis=1)
        # [P, NT, K] partition-major -> [T, K] token-major (t = ti*128 + p)
        idx_parts.append(
            np.transpose(idx_full, (1, 0, 2)).reshape(T, TOPK).astype(np.int32)
        )
        w_parts.append(np.transpose(w_full, (1, 0, 2)).reshape(T, TOPK))
        probs_parts.append(aux[0, :E])
        cnt_parts.append(aux[1, E:])

    topk_idx = np.concatenate(idx_parts, axis=0)
    topk_w = np.ascontiguousarray(np.concatenate(w_parts, axis=0))

    # two cores per batch row
    probs = np.stack(probs_parts).reshape(B, 2, E).sum(axis=1, dtype=np.float32)
    cnt = np.stack(cnt_parts).reshape(B, 2, E).sum(axis=1, dtype=np.float32)
    probs = probs / (probs.sum(axis=-1, keepdims=True) + np.float32(1e-9))
    cnt = cnt / (cnt.sum(axis=-1, keepdims=True) + np.float32(1e-9))
    aux = (probs * cnt).sum(axis=-1).mean() * np.float32(E * 0.01)
    return topk_idx, topk_w, np.float32(aux)


def kernel(x, W_gate):
    nc = _get_nc()
    in_maps = _make_in_maps(x, W_gate)
    res = run_bass_kernel_spmd(nc, in_maps, list(range(N_CORES))).results
    return _finalize(res)
